# revision 1
# baseline (speedup 1.0000x reference)
"""GAT Trainium kernel: host prep + Bass/Tile builder + runner.

Sharding: nodes by graph blocks (graphs 8d..8d+8 -> device d), each device
owns all edges whose dst lands in its node range. Per layer:
  - feature-major matmuls h^T = W^T @ x^T, attention logit rows al^T
  - PE transposes -> node-major staging table [h | u v a b | pad] (320 f32)
  - AllGather table -> [8*NPAD, 320]
  - per 128-node block: dma_gather src rows (1280B) + dst rows (256B),
    p = max(u_src*a_dst, v_src*b_dst)  (exp(leaky(ls+ld)) factorized),
    messages p*h, one-hot scatter matmul into PSUM -> U | S,
    out = U/(S+1e-16) + b
  - BN stats via ones-matmul + AllReduce, apply + relu on node-major tiles
  - pooling via one-hot graph matmul; MLP replicated on gathered pooled.
"""
import contextlib
import numpy as np

import concourse.bass as bass
import concourse.tile as tile
from concourse import bacc, mybir
from concourse.bass_utils import run_bass_kernel_spmd

F32 = mybir.dt.float32
I16 = mybir.dt.int16
AF = mybir.ActivationFunctionType
OP = mybir.AluOpType

N, E, G = 20000, 320000, 64
H, C = 4, 64
DIN, HID, LAT = 128, 256, 64
EPS = 1e-5
NCORES = 8
GPD = G // NCORES  # graphs per device


def _pack16(idx_2d):
    """[nblk, S*128] linear order -> [128, nblk*S*8] int16 wrapped layout."""
    nblk, L = idx_2d.shape
    out = np.zeros((128, nblk * (L // 16)), np.int16)
    for b in range(nblk):
        a = idx_2d[b].reshape(L // 16, 16).T  # [16, L/16]
        out[:, b * (L // 16):(b + 1) * (L // 16)] = np.tile(a, (8, 1))
    return out


def host_prep(inputs):
    """Build per-device arrays. Returns (meta, in_maps)."""
    x = np.asarray(inputs['x'], np.float32)
    ei = np.asarray(inputs['edge_index'], np.int32)
    batch = np.asarray(inputs['batch'], np.int32)

    src = np.concatenate([ei[0], np.arange(N, dtype=np.int32)])
    dst = np.concatenate([ei[1], np.arange(N, dtype=np.int32)])

    gsize = np.bincount(batch, minlength=G)
    dev_cnt = gsize.reshape(NCORES, GPD).sum(1)
    starts = np.zeros(NCORES, np.int64)
    starts[1:] = np.cumsum(dev_cnt)[:-1]
    NPAD = int(np.ceil(dev_cnt.max() / 128) * 128)
    NBLK = NPAD // 128
    assert NCORES * NPAD < 32768  # int16 gather indices

    node_dev = np.repeat(np.arange(NCORES), dev_cnt)          # [N]
    node_loc = np.arange(N) - starts[node_dev]                # [N]
    node_row = (node_dev * NPAD + node_loc).astype(np.int64)  # global table row
    src_row = node_row[src]

    # per-device edge layout
    SUB = 0
    per_dev = []
    for d in range(NCORES):
        m = (dst >= starts[d]) & (dst < starts[d] + dev_cnt[d])
        dl = (dst[m] - starts[d]).astype(np.int64)
        order = np.argsort(dl, kind='stable')
        dl = dl[order]
        sr = src_row[m][order]
        cnt_b = np.bincount(dl // 128, minlength=NBLK)
        SUB = max(SUB, int(np.ceil(cnt_b.max() / 128)))
        per_dev.append((dl, sr, cnt_b, d))
    L = SUB * 128

    def dev_arrays(dl, sr, cnt_b, d):
        src_sl = np.zeros((NBLK, L), np.int64)
        dst_sl = np.zeros((NBLK, L), np.int64)
        win = np.full((NBLK, L), -1.0, np.float32)
        off = 0
        for b in range(NBLK):
            nb = int(cnt_b[b])
            src_sl[b, :nb] = sr[off:off + nb]
            dst_sl[b, :nb] = d * NPAD + dl[off:off + nb]
            win[b, :nb] = (dl[off:off + nb] - 128 * b).astype(np.float32)
            off += nb
        # win packed: partition = pos in subtile, col = b*SUB + j
        winp = win.reshape(NBLK, SUB, 128).transpose(2, 0, 1).reshape(
            128, NBLK * SUB).copy()
        return (_pack16(src_sl), _pack16(dst_sl), winp)

    gbcol = np.zeros((128, 2, 3, 2), np.float32)  # [p, ft, layer, g/beta]
    for lyr in range(3):
        g_ = np.asarray(inputs[f'bn_g{lyr}'], np.float32)
        be_ = np.asarray(inputs[f'bn_b{lyr}'], np.float32)
        for ft in range(2):
            gbcol[:, ft, lyr, 0] = g_[ft * 128:(ft + 1) * 128]
            gbcol[:, ft, lyr, 1] = be_[ft * 128:(ft + 1) * 128]
    iota = np.tile(np.arange(128, dtype=np.float32)[None, :], (128, 1))
    ident = np.eye(128, dtype=np.float32)
    onescol = np.ones((128, 1), np.float32)
    onesrow = np.ones((1, 128), np.float32)

    def f32(a):
        return np.ascontiguousarray(a, dtype=np.float32)

    W0 = f32(inputs['W0'])                      # [128, 256]
    W12 = np.stack([f32(inputs['W1']).reshape(2, 128, HID),
                    f32(inputs['W2']).reshape(2, 128, HID)])  # [2,2,128,256]
    AW = np.zeros((3, 2, 128, 8), np.float32)
    for lyr in range(3):
        a_s = f32(inputs[f'a_src{lyr}'])
        a_d = f32(inputs[f'a_dst{lyr}'])
        A = np.zeros((HID, 8), np.float32)
        for h in range(H):
            A[h * C:(h + 1) * C, h] = a_s[h]
            A[h * C:(h + 1) * C, 4 + h] = a_d[h]
        AW[lyr] = A.reshape(2, 128, 8)
    BBC = np.stack([np.tile(f32(inputs[f'b{lyr}'])[None, :], (128, 1))
                    for lyr in range(3)])       # [3,128,256]
    GB = np.stack([np.concatenate([f32(inputs[f'bn_g{lyr}']),
                                   f32(inputs[f'bn_b{lyr}'])])
                   for lyr in range(3)])        # [3,512]
    MW1 = f32(inputs['mw1']).reshape(2, 128, HID)
    MW2 = f32(inputs['mw2']).reshape(2, 128, HID)
    MW3 = f32(inputs['mw3']).reshape(2, 128, LAT)
    MB1 = np.tile(f32(inputs['mb1'])[None, :], (G, 1))
    MB2 = np.tile(f32(inputs['mb2'])[None, :], (G, 1))
    MB3 = np.tile(f32(inputs['mb3'])[None, :], (G, 1))
    MGB1 = np.concatenate([f32(inputs['mg1']), f32(inputs['mbeta1'])])[None, :]
    MGB2 = np.concatenate([f32(inputs['mg2']), f32(inputs['mbeta2'])])[None, :]

    in_maps = []
    for d in range(NCORES):
        dl, sr, cnt_b, _ = per_dev[d]
        sidx, didx, winp = dev_arrays(dl, sr, cnt_b, d)
        xs = np.zeros((NPAD, DIN), np.float32)
        xs[:dev_cnt[d]] = x[starts[d]:starts[d] + dev_cnt[d]]
        pad_d = float(NPAD - dev_cnt[d])
        corr = np.zeros((3, 2 * HID), np.float32)
        for lyr in range(3):
            bl = f32(inputs[f'b{lyr}'])
            corr[lyr, :HID] = pad_d * bl
            corr[lyr, HID:] = pad_d * bl * bl
        gsel = np.zeros((NPAD, GPD), np.float32)
        gloc = batch[starts[d]:starts[d] + dev_cnt[d]] - d * GPD
        gsel[np.arange(dev_cnt[d]), gloc] = 1.0
        gselp = gsel.reshape(NBLK, 128, GPD).transpose(1, 0, 2).reshape(
            128, NBLK * GPD).copy()
        in_maps.append(dict(
            xT=np.ascontiguousarray(xs.T), src_idx=sidx, dst_idx=didx,
            dstwin=winp, gsel=gselp, iota=iota, ident=ident, GBCOL=gbcol,
            onescol=onescol, onesrow=onesrow,
            W0=W0, W12=W12, AW=AW, BBC=BBC, CORR=corr, GB=GB,
            MW1=MW1, MW2=MW2, MW3=MW3, MB1=MB1, MB2=MB2, MB3=MB3,
            MGB1=MGB1, MGB2=MGB2,
        ))
    meta = dict(NPAD=NPAD, NBLK=NBLK, SUB=SUB)
    return meta, in_maps


def build_nc(meta, repeat=1, no_gather=False, no_scatter=False, no_coll=False):
    NPAD, NBLK, SUB = meta['NPAD'], meta['NBLK'], meta['SUB']
    TROW = 320
    ROWS = NCORES * NPAD
    NCHUNK = NPAD // 512 if NPAD % 512 == 0 else None
    assert NPAD % 512 == 0, NPAD

    nc = bacc.Bacc("TRN2", target_bir_lowering=False, debug=False,
                   num_devices=NCORES)

    def din(name, shape, dt=F32):
        return nc.dram_tensor(name, shape, dt, kind="ExternalInput")

    xT = din("xT", [DIN, NPAD])
    src_idx = din("src_idx", [128, NBLK * SUB * 8], I16)
    dst_idx = din("dst_idx", [128, NBLK * SUB * 8], I16)
    dstwin = din("dstwin", [128, NBLK * SUB])
    gsel = din("gsel", [128, NBLK * GPD])
    iota = din("iota", [128, 128])
    ident = din("ident", [128, 128])
    onescol = din("onescol", [128, 1])
    onesrow = din("onesrow", [1, 128])
    W0 = din("W0", [128, HID])
    W12 = din("W12", [2, 2, 128, HID])
    AW = din("AW", [3, 2, 128, 8])
    BBC = din("BBC", [3, 128, HID])
    CORR = din("CORR", [3, 2 * HID])
    GB = din("GB", [3, 2 * HID])
    GBCOL = din("GBCOL", [128, 2, 3, 2])
    MW1 = din("MW1", [2, 128, HID])
    MW2 = din("MW2", [2, 128, HID])
    MW3 = din("MW3", [2, 128, LAT])
    MB1 = din("MB1", [G, HID])
    MB2 = din("MB2", [G, HID])
    MB3 = din("MB3", [G, LAT])
    MGB1 = din("MGB1", [1, 2 * HID])
    MGB2 = din("MGB2", [1, 2 * HID])
    out_t = nc.dram_tensor("out", [G, LAT], F32, kind="ExternalOutput")

    with tile.TileContext(nc) as tc:
        ctx = contextlib.ExitStack()
        pers = ctx.enter_context(tc.tile_pool(name="pers", bufs=1))
        work = ctx.enter_context(tc.tile_pool(name="work", bufs=2))
        gwork = ctx.enter_context(tc.tile_pool(name="gwork", bufs=2))
        psum = ctx.enter_context(tc.tile_pool(name="psum", bufs=2, space="PSUM"))
        psacc = ctx.enter_context(tc.tile_pool(name="psacc", bufs=1, space="PSUM"))
        dram = ctx.enter_context(tc.tile_pool(name="dram", bufs=1, space="DRAM"))

        def load(ap, shape, dt=F32, pool=pers, name=None):
            t = pool.tile(shape, dt, name=name or ap.tensor.name + "_sb")
            nc.sync.dma_start(out=t[:], in_=ap)
            return t

        # ---- persistent constants in SBUF
        iota_t = load(iota[:], [128, 128])
        ident_t = load(ident[:], [128, 128])
        onescol_t = load(onescol[:], [128, 1])
        onesrow_t = load(onesrow[:], [1, 128])
        sidx_t = load(src_idx[:], [128, NBLK * SUB * 8], I16)
        didx_t = load(dst_idx[:], [128, NBLK * SUB * 8], I16)
        win_t = load(dstwin[:], [128, NBLK * SUB])
        gsel_t = load(gsel[:], [128, NBLK * GPD])
        W0_t = load(W0[:], [128, HID])
        W12_t = load(W12[:].rearrange('a b p c -> p a b c'), [128, 2, 2, HID])
        AW_t = load(AW[:].rearrange('l k p c -> p l k c'), [128, 3, 2, 8])
        BBC_t = load(BBC[:].rearrange('l p c -> p l c'), [128, 3, HID])
        CORR_t = load(CORR[:].rearrange('(x l) c -> x l c', x=1), [1, 3, 2 * HID])
        GB_t = load(GB[:].rearrange('(x l) c -> x l c', x=1), [1, 3, 2 * HID])
        GBCOL_t = load(GBCOL[:], [128, 2, 3, 2])
        MW1_t = load(MW1[:].rearrange('k p c -> p k c'), [128, 2, HID])
        MW2_t = load(MW2[:].rearrange('k p c -> p k c'), [128, 2, HID])
        MW3_t = load(MW3[:].rearrange('k p c -> p k c'), [128, 2, LAT])
        MB1_t = load(MB1[:], [G, HID])
        MB2_t = load(MB2[:], [G, HID])
        MB3_t = load(MB3[:], [G, LAT])
        MGB1_t = load(MGB1[:], [1, 2 * HID])
        MGB2_t = load(MGB2[:], [1, 2 * HID])
        # persistent stages
        hTa = pers.tile([128, 2, NPAD], F32, name="hTa")
        hTb = pers.tile([128, 2, NPAD], F32, name="hTb")
        nc.sync.dma_start(out=hTa[:, 0, :], in_=xT[:])
        STW = 272  # staged table row (written part; DRAM stride stays TROW)
        stage = pers.tile([128, NBLK, STW], F32, name="stage")
        alstg = pers.tile([128, NBLK, 8], F32, name="alstg")

        loc_table = dram.tile([NPAD, TROW], F32, name="loc_table")
        gat_tables = [dram.tile([ROWS, TROW], F32, addr_space="Shared",
                                name=f"gat_table{i}")
                      for i in range(3 * repeat)]
        abloc = dram.tile([NPAD, 64], F32, name="abloc")
        abgats = [dram.tile([ROWS, 64], F32, addr_space="Shared",
                            name=f"abgat{i}") for i in range(3 * repeat)]
        stats_loc = dram.tile([1, 2 * HID], F32, name="stats_loc")
        stats_shs = [dram.tile([1, 2 * HID], F32, addr_space="Shared",
                               name=f"stats_sh{i}") for i in range(3 * repeat)]
        pool_loc = dram.tile([GPD, HID], F32, name="pool_loc")
        pool_shs = [dram.tile([G, HID], F32, addr_space="Shared",
                              name=f"pool_sh{i}") for i in range(repeat)]

        RG = [list(range(NCORES))]

        def bcast_mid(ap, mid, inner):
            return ap.rearrange('p (x f) -> p x f', x=1).to_broadcast(
                [128, mid, inner])

        for _rep in range(repeat):
            hcur, hnxt = hTa, hTb
            for L in range(3):
                KT = 1 if L == 0 else 2
                # ---- 1. layer matmul: hnxt[ft] = sum_kt W^T x
                for ft in range(2):
                    for ch in range(NPAD // 512):
                        ph = psum.tile([128, 512], F32, name="ph", tag="ps", bufs=3)
                        for kt in range(KT):
                            if L == 0:
                                lhsT = W0_t[:, ft * 128:(ft + 1) * 128]
                                rhs = hcur[:, 0, ch * 512:(ch + 1) * 512]
                            else:
                                lhsT = W12_t[:, L - 1, kt, ft * 128:(ft + 1) * 128]
                                rhs = hcur[:, kt, ch * 512:(ch + 1) * 512]
                            nc.tensor.matmul(out=ph[:], lhsT=lhsT, rhs=rhs,
                                             start=(kt == 0), stop=(kt == KT - 1))
                        nc.vector.tensor_copy(
                            out=hnxt[:, ft, ch * 512:(ch + 1) * 512], in_=ph[:])
                # ---- 2. al^T [8, NPAD]
                for ch in range(NPAD // 512):
                    pa = psum.tile([8, 512], F32, name="pa", tag="ps", bufs=3)
                    for kt in range(2):
                        nc.tensor.matmul(
                            out=pa[:], lhsT=AW_t[:, L, kt, :],
                            rhs=hnxt[:, kt, ch * 512:(ch + 1) * 512],
                            start=(kt == 0), stop=(kt == 1))
                    alc = work.tile([8, 512], F32, name="alc")
                    nc.vector.tensor_copy(out=alc[:], in_=pa[:])
                    pal = psum.tile([128, 32], F32, name="pal", tag="ps", bufs=3)
                    for q in range(4):
                        nc.tensor.transpose(
                            out=pal[:, q * 8:(q + 1) * 8],
                            in_=alc[:, q * 128:(q + 1) * 128],
                            identity=ident_t[0:8, 0:8])
                    nc.vector.tensor_copy(out=alstg[:, ch * 4:ch * 4 + 4, :],
                                          in_=pal[:])
                # ---- 3. transposes to node-major staging (packed)
                for tp in range(NBLK // 2):
                    pt = psum.tile([128, 512], F32, name="pt", tag="ps", bufs=3)
                    for q in range(4):
                        t_i, ft = tp * 2 + q // 2, q % 2
                        nc.tensor.transpose(
                            out=pt[:, q * 128:(q + 1) * 128],
                            in_=hnxt[:, ft, t_i * 128:(t_i + 1) * 128],
                            identity=ident_t[:])
                    nc.vector.tensor_copy(
                        out=stage[:, tp * 2:tp * 2 + 2, 0:256], in_=pt[:])
                # ---- 4. exp -> u v a b  (cols 256:272)
                nc.scalar.activation(stage[:, :, 256:260], alstg[:, :, 0:4],
                                     AF.Exp)
                nc.scalar.activation(stage[:, :, 260:264], alstg[:, :, 0:4],
                                     AF.Exp, scale=0.2)
                nc.scalar.activation(stage[:, :, 264:268], alstg[:, :, 4:8],
                                     AF.Exp)
                nc.scalar.activation(stage[:, :, 268:272], alstg[:, :, 4:8],
                                     AF.Exp, scale=0.2)
                gat_table = gat_tables[_rep * 3 + L]
                abgat = abgats[_rep * 3 + L]
                stats_sh = stats_shs[_rep * 3 + L]
                # ---- 5. write local table + AllGather
                nc.sync.dma_start(
                    out=loc_table[:, 0:STW].rearrange('(t p) c -> p t c',
                                                      p=128),
                    in_=stage[:, :, :])
                nc.sync.dma_start(
                    out=abloc[:, 0:8].rearrange('(t p) c -> p t c', p=128),
                    in_=stage[:, :, 264:272])
                if no_coll:
                    nc.sync.dma_start(out=gat_table[0:NPAD, :],
                                      in_=loc_table[:])
                    nc.sync.dma_start(out=abgat[0:NPAD, :], in_=abloc[:])
                else:
                    nc.gpsimd.collective_compute(
                        "AllGather", OP.bypass, replica_groups=RG,
                        ins=[loc_table[:]], outs=[gat_table[:]])
                    nc.gpsimd.collective_compute(
                        "AllGather", OP.bypass, replica_groups=RG,
                        ins=[abloc[:]], outs=[abgat[:]])  # noqa
                # ---- 6. edge phase
                sum_ps = psacc.tile([1, 2 * HID], F32, name="sum_ps")
                sq_ps = psacc.tile([1, 2 * HID], F32, name="sq_ps")
                for b in range(NBLK):
                    srcg = gwork.tile([128, SUB, TROW], F32, name="srcg")
                    dstg = gwork.tile([128, SUB, 64], F32, name="dstg", bufs=1)
                    if no_gather:
                        nc.vector.memset(srcg[:], 0.5)
                        nc.vector.memset(dstg[:], 0.5)
                    else:
                        nc.gpsimd.dma_gather(
                            srcg[:], gat_table[:],
                            sidx_t[:, b * SUB * 8:(b + 1) * SUB * 8],
                            SUB * 128, SUB * 128, TROW, single_packet=False)
                        nc.gpsimd.dma_gather(
                            dstg[:], abgat[:],
                            didx_t[:, b * SUB * 8:(b + 1) * SUB * 8],
                            SUB * 128, SUB * 128, 64, single_packet=False)
                    mt = gwork.tile([128, SUB, 128], F32, name="mt", bufs=1)
                    nc.vector.tensor_tensor(
                        out=mt[:], in0=bcast_mid(iota_t[:], SUB, 128),
                        in1=win_t[:, b * SUB:(b + 1) * SUB].rearrange(
                            'p (s x) -> p s x', x=1).to_broadcast(
                            [128, SUB, 128]),
                        op=OP.is_equal)
                    t12 = work.tile([128, SUB, 8], F32, name="t12")
                    nc.vector.tensor_tensor(out=t12[:],
                                            in0=srcg[:, :, 256:264],
                                            in1=dstg[:, :, 0:8], op=OP.mult)
                    nc.vector.tensor_tensor(out=srcg[:, :, 256:260],
                                            in0=t12[:, :, 0:4],
                                            in1=t12[:, :, 4:8], op=OP.max)
                    nc.vector.tensor_tensor(
                        out=srcg[:, :, 0:256].rearrange(
                            'p s (h c) -> p s h c', c=64),
                        in0=srcg[:, :, 0:256].rearrange(
                            'p s (h c) -> p s h c', c=64),
                        in1=srcg[:, :, 256:260].rearrange(
                            'p s (h x) -> p s h x', x=1).to_broadcast(
                            [128, SUB, 4, 64]),
                        op=OP.mult)
                    blkp = psum.tile([128, 260], F32, name="blkp")
                    nsc = 1 if no_scatter else SUB
                    for j in range(nsc):
                        nc.tensor.matmul(out=blkp[:], lhsT=mt[:, j, :],
                                         rhs=srcg[:, j, 0:260],
                                         start=(j == 0), stop=(j == nsc - 1))
                    srec = work.tile([128, 4], F32, name="srec")
                    nc.vector.tensor_scalar_add(srec[:], blkp[:, 256:260],
                                                1e-16)
                    nc.vector.reciprocal(srec[:], srec[:])
                    nc.vector.tensor_tensor(
                        out=stage[:, b, 0:256].rearrange('p (h c) -> p h c', c=64),
                        in0=blkp[:, 0:256].rearrange('p (h c) -> p h c', c=64),
                        in1=srec[:].rearrange('p (h x) -> p h x',
                                              x=1).to_broadcast([128, 4, 64]),
                        op=OP.mult)
                    nc.vector.tensor_tensor(out=stage[:, b, 0:256],
                                            in0=stage[:, b, 0:256],
                                            in1=BBC_t[:, L, :], op=OP.add)
                assert NBLK % 2 == 0
                for bb in range(NBLK // 2):
                    sq = work.tile([128, 512], F32, name="sq", bufs=1)
                    nc.scalar.activation(sq[:],
                                         stage[:, 2 * bb:2 * bb + 2, 0:256],
                                         AF.Square)
                    nc.tensor.matmul(out=sum_ps[:], lhsT=onescol_t[:],
                                     rhs=stage[:, 2 * bb:2 * bb + 2, 0:256],
                                     start=(bb == 0),
                                     stop=(bb == NBLK // 2 - 1))
                    nc.tensor.matmul(out=sq_ps[:], lhsT=onescol_t[:],
                                     rhs=sq[:], start=(bb == 0),
                                     stop=(bb == NBLK // 2 - 1))
                # ---- 7. stats AllReduce
                st4 = work.tile([1, 4 * HID], F32, name="st4", bufs=1)
                nc.vector.tensor_copy(out=st4[:, 0:2 * HID], in_=sum_ps[:])
                nc.vector.tensor_copy(out=st4[:, 2 * HID:], in_=sq_ps[:])
                sstat = work.tile([1, 2 * HID], F32, name="sstat", bufs=1)
                nc.vector.tensor_tensor(
                    out=sstat[:].rearrange('p (a c) -> p a c', a=2),
                    in0=st4[:].rearrange('p (a b c) -> p a b c', a=2, b=2
                                         )[:, :, 0, :],
                    in1=st4[:].rearrange('p (a b c) -> p a b c', a=2, b=2
                                         )[:, :, 1, :],
                    op=OP.add)
                nc.vector.tensor_tensor(out=sstat[:], in0=sstat[:],
                                        in1=CORR_t[:, L, :], op=OP.subtract)
                nc.sync.dma_start(out=stats_loc[:], in_=sstat[:])
                if no_coll:
                    nc.sync.dma_start(out=stats_sh[:], in_=stats_loc[:])
                else:
                    nc.gpsimd.collective_compute(
                        "AllReduce", OP.add, replica_groups=RG,
                        ins=[stats_loc[:]], outs=[stats_sh[:]])
                sar = work.tile([1, 2 * HID], F32, name="sar", bufs=1)
                nc.sync.dma_start(out=sar[:], in_=stats_sh[:])
                if L < 2:
                    # ---- 8/9/10 fused: column-form bn coefs; transpose
                    # pre-bn stage -> hcur; fused scale+bias+relu on hcur.
                    pscl = psum.tile([128, 4], F32, name="pscl", tag="ps",
                                     bufs=3)
                    for q in range(4):
                        nc.tensor.transpose(
                            out=pscl[:, q:q + 1],
                            in_=sar[:, q * 128:(q + 1) * 128],
                            identity=ident_t[0:1, 0:1])
                    scol = work.tile([128, 4], F32, name="scol", bufs=1)
                    nc.vector.tensor_copy(out=scol[:], in_=pscl[:])
                    meanc = work.tile([128, 2], F32, name="meanc", bufs=1)
                    nc.scalar.activation(meanc[:], scol[:, 0:2], AF.Copy,
                                         scale=1.0 / N)
                    msqc = work.tile([128, 2], F32, name="msqc", bufs=1)
                    nc.scalar.activation(msqc[:], meanc[:], AF.Square)
                    varc = work.tile([128, 2], F32, name="varc", bufs=1)
                    nc.scalar.activation(varc[:], scol[:, 2:4], AF.Copy,
                                         scale=1.0 / N)
                    nc.vector.tensor_tensor(out=varc[:], in0=varc[:],
                                            in1=msqc[:], op=OP.subtract)
                    nc.vector.tensor_scalar_add(varc[:], varc[:], EPS)
                    nc.vector.reciprocal(varc[:], varc[:])
                    rstdc = work.tile([128, 2], F32, name="rstdc", bufs=1)
                    nc.scalar.activation(rstdc[:], varc[:], AF.Sqrt)
                    scoef = work.tile([128, 2], F32, name="scoef", bufs=1)
                    nc.vector.tensor_tensor(out=scoef[:],
                                            in0=GBCOL_t[:, :, L, 0],
                                            in1=rstdc[:], op=OP.mult)
                    ocoef = work.tile([128, 2], F32, name="ocoef", bufs=1)
                    nc.vector.tensor_tensor(out=ocoef[:], in0=meanc[:],
                                            in1=scoef[:], op=OP.mult)
                    nc.vector.tensor_tensor(out=ocoef[:],
                                            in0=GBCOL_t[:, :, L, 1],
                                            in1=ocoef[:], op=OP.subtract)
                    for tp in range(NBLK // 2):
                        pt2 = psum.tile([128, 512], F32, name="pt2",
                                        tag="ps", bufs=3)
                        for q in range(4):
                            t_i, ft = tp * 2 + q // 2, q % 2
                            nc.tensor.transpose(
                                out=pt2[:, q * 128:(q + 1) * 128],
                                in_=stage[:, t_i, ft * 128:(ft + 1) * 128],
                                identity=ident_t[:])
                        nc.vector.tensor_copy(
                            out=hcur[:, :, tp * 256:(tp + 1) * 256].rearrange(
                                'p f (t x) -> p t f x', t=2),
                            in_=pt2[:])
                    for ft in range(2):
                        nc.scalar.activation(hcur[:, ft, :], hcur[:, ft, :],
                                             AF.Relu,
                                             bias=ocoef[:, ft:ft + 1],
                                             scale=scoef[:, ft:ft + 1])
                else:
                    # ---- 8. bn coefficients (row form, node-major apply)
                    mrow = work.tile([1, HID], F32, name="mrow", bufs=1)
                    nc.scalar.activation(mrow[:], sar[:, 0:HID], AF.Copy,
                                         scale=1.0 / N)
                    msq = work.tile([1, HID], F32, name="msq", bufs=1)
                    nc.scalar.activation(msq[:], mrow[:], AF.Square)
                    vrow = work.tile([1, HID], F32, name="vrow", bufs=1)
                    nc.scalar.activation(vrow[:], sar[:, HID:], AF.Copy,
                                         scale=1.0 / N)
                    nc.vector.tensor_tensor(out=vrow[:], in0=vrow[:],
                                            in1=msq[:], op=OP.subtract)
                    nc.vector.tensor_scalar_add(vrow[:], vrow[:], EPS)
                    nc.vector.reciprocal(vrow[:], vrow[:])
                    rstd = work.tile([1, HID], F32, name="rstd", bufs=1)
                    nc.scalar.activation(rstd[:], vrow[:], AF.Sqrt)
                    so = work.tile([1, 2 * HID], F32, name="so", bufs=1)
                    nc.vector.tensor_tensor(out=so[:, 0:HID],
                                            in0=GB_t[:, L, 0:HID],
                                            in1=rstd[:], op=OP.mult)
                    nc.vector.tensor_tensor(out=so[:, HID:], in0=mrow[:],
                                            in1=so[:, 0:HID], op=OP.mult)
                    nc.vector.tensor_tensor(out=so[:, HID:],
                                            in0=GB_t[:, L, HID:],
                                            in1=so[:, HID:], op=OP.subtract)
                    pso = psum.tile([128, 2 * HID], F32, name="pso", tag="ps",
                                    bufs=3)
                    nc.tensor.matmul(out=pso[:], lhsT=onesrow_t[:], rhs=so[:],
                                     start=True, stop=True)
                    mso = work.tile([128, 2 * HID], F32, name="mso", bufs=1)
                    nc.vector.tensor_copy(out=mso[:], in_=pso[:])
                    for b in range(NBLK):
                        nc.vector.tensor_tensor(out=stage[:, b, 0:256],
                                                in0=stage[:, b, 0:256],
                                                in1=mso[:, 0:HID], op=OP.mult)
                        nc.vector.tensor_tensor(out=stage[:, b, 0:256],
                                                in0=stage[:, b, 0:256],
                                                in1=mso[:, HID:], op=OP.add)
                        nc.scalar.activation(stage[:, b, 0:256],
                                             stage[:, b, 0:256], AF.Relu)
                    poolp = psacc.tile([GPD, HID], F32, name="poolp")
                    for b in range(NBLK):
                        nc.tensor.matmul(
                            out=poolp[:],
                            lhsT=gsel_t[:, b * GPD:(b + 1) * GPD],
                            rhs=stage[:, b, 0:256], start=(b == 0),
                            stop=(b == NBLK - 1))
                    pooled = work.tile([GPD, HID], F32, name="pooled", bufs=1)
                    nc.vector.tensor_copy(out=pooled[:], in_=poolp[:])
                    nc.sync.dma_start(out=pool_loc[:], in_=pooled[:])
                    if no_coll:
                        nc.sync.dma_start(out=pool_shs[_rep][0:GPD, :],
                                          in_=pool_loc[:])
                    else:
                        nc.gpsimd.collective_compute(
                            "AllGather", OP.bypass, replica_groups=RG,
                            ins=[pool_loc[:]], outs=[pool_shs[_rep][:]])

            # ---- MLP (replicated over all 64 graphs)
            cur = work.tile([G, HID], F32, name="mlp_cur", bufs=1)
            nc.sync.dma_start(out=cur[:], in_=pool_shs[_rep][:])
            for li, (MWt, MBt, MGBt, dout) in enumerate([
                    (MW1_t, MB1_t, MGB1_t, HID),
                    (MW2_t, MB2_t, MGB2_t, HID),
                    (MW3_t, MB3_t, None, LAT)]):
                curT = work.tile([128, 2, G], F32, name="curT")
                for kt in range(2):
                    ptm = psum.tile([128, G], F32, name="ptm", tag="ps", bufs=3)
                    nc.tensor.transpose(out=ptm[:],
                                        in_=cur[:, kt * 128:(kt + 1) * 128],
                                        identity=ident_t[0:G, 0:G])
                    nc.vector.tensor_copy(out=curT[:, kt, :], in_=ptm[:])
                pm = psum.tile([G, dout], F32, name="pm", tag="ps", bufs=3)
                for kt in range(2):
                    nc.tensor.matmul(out=pm[:], lhsT=curT[:, kt, :], rhs=MWt[:, kt, :],
                                     start=(kt == 0), stop=(kt == 1))
                nxt = work.tile([G, dout], F32, name=f"mlp_o{li}", bufs=1)
                nc.vector.tensor_tensor(out=nxt[:], in0=pm[:], in1=MBt[:],
                                        op=OP.add)
                lk = work.tile([G, dout], F32, name="mlp_lk", tag="mlp_lk", bufs=1)
                nc.vector.tensor_scalar_mul(lk[:], nxt[:], 0.2)
                nc.vector.tensor_tensor(out=nxt[:], in0=nxt[:], in1=lk[:],
                                        op=OP.max)
                if MGBt is not None:
                    spm = psum.tile([1, dout], F32, name="spm", tag="ps", bufs=3)
                    nc.tensor.matmul(out=spm[:], lhsT=onescol_t[0:G, :],
                                     rhs=nxt[:], start=True, stop=True)
                    sqm = work.tile([G, dout], F32, name="sqm", bufs=1)
                    nc.scalar.activation(sqm[:], nxt[:], AF.Square)
                    sqpm = psum.tile([1, dout], F32, name="sqpm", tag="ps", bufs=3)
                    nc.tensor.matmul(out=sqpm[:], lhsT=onescol_t[0:G, :],
                                     rhs=sqm[:], start=True, stop=True)
                    mrow2 = work.tile([1, dout], F32, name="mrow2", bufs=1)
                    nc.scalar.activation(mrow2[:], spm[:], AF.Copy,
                                         scale=1.0 / G)
                    msq2 = work.tile([1, dout], F32, name="msq2", bufs=1)
                    nc.scalar.activation(msq2[:], mrow2[:], AF.Square)
                    vrow2 = work.tile([1, dout], F32, name="vrow2", bufs=1)
                    nc.scalar.activation(vrow2[:], sqpm[:], AF.Copy,
                                         scale=1.0 / G)
                    nc.vector.tensor_tensor(out=vrow2[:], in0=vrow2[:],
                                            in1=msq2[:], op=OP.subtract)
                    nc.vector.tensor_scalar_add(vrow2[:], vrow2[:], EPS)
                    nc.vector.reciprocal(vrow2[:], vrow2[:])
                    rstd2 = work.tile([1, dout], F32, name="rstd2", bufs=1)
                    nc.scalar.activation(rstd2[:], vrow2[:], AF.Sqrt)
                    so2 = work.tile([1, 2 * dout], F32, name="so2", bufs=1)
                    nc.vector.tensor_tensor(out=so2[:, 0:dout],
                                            in0=MGBt[:, 0:dout], in1=rstd2[:],
                                            op=OP.mult)
                    nc.vector.tensor_tensor(out=so2[:, dout:], in0=mrow2[:],
                                            in1=so2[:, 0:dout], op=OP.mult)
                    nc.vector.tensor_tensor(out=so2[:, dout:],
                                            in0=MGBt[:, dout:2 * dout],
                                            in1=so2[:, dout:],
                                            op=OP.subtract)
                    pso2 = psum.tile([G, 2 * dout], F32, name="pso2", tag="ps", bufs=3)
                    nc.tensor.matmul(out=pso2[:], lhsT=onesrow_t[:, 0:G],
                                     rhs=so2[:], start=True, stop=True)
                    mso2 = work.tile([G, 2 * dout], F32, name="mso2", bufs=1)
                    nc.vector.tensor_copy(out=mso2[:], in_=pso2[:])
                    nc.vector.tensor_tensor(out=nxt[:], in0=nxt[:],
                                            in1=mso2[:, 0:dout], op=OP.mult)
                    nc.vector.tensor_tensor(out=nxt[:], in0=nxt[:],
                                            in1=mso2[:, dout:], op=OP.add)
                cur = nxt
            nc.sync.dma_start(out=out_t[:], in_=cur[:])
        ctx.close()

    nc.compile()
    return nc


_CACHE = {}


def kernel(**inputs):
    meta, in_maps = host_prep(inputs)
    key = tuple(sorted(meta.items()))
    if key not in _CACHE:
        _CACHE[key] = build_nc(meta)
    nc = _CACHE[key]
    res = run_bass_kernel_spmd(nc, in_maps, list(range(NCORES)))
    return res.results[0]["out"]
